# revision 20
# baseline (speedup 1.0000x reference)
"""Trainium2 Bass kernel for a GRU language model.

  logits[b,s,:] = Dense(GRU(Embed(x)))   (keras GRU, reset_after=True)

Shapes: x[16,256] i32, E[32000,256], W[256,1536], U[512,1536], b[2,1536],
Wd[512,32000], bd[32000] -> logits [16,256,32000] f32.

Strategy (8 cores):
  * Tensor-parallel over the vocab dim of Wd (4000 cols/core); the GRU
    recurrence is replicated on every core (it is serial in S and cheap
    compared to the 524MB logits write).  No collectives.
  * s-major token order: token t' = s*16 + b.  Each "group" g covers 128
    tokens = 8 timesteps x 16 batch.
  * Embedding gather via indirect DMA (128 rows/descriptor-batch).
  * xw = emb @ W precomputed per group on PE (embT built once via PE
    transposes); folded into the per-step PSUM with an identity matmul.
  * Recurrence h @ U: the hidden state is kept in DVE 32x32
    block-transposed form (bt), replicated to the 4 partition row-groups,
    so each step's matmuls are 4-way row-tiled (K=32 tiles at partition
    offsets 0/32/64/96) which the PE runs concurrently.
  * All matmul operands bf16, accumulation fp32 in PSUM.
  * Output projection uses the same row-tiled trick straight out of the
    stored bt tiles (lhsT = bt[32i:32i+32, 0:8, 32b:32b+16]), N-chunks of
    500 (one PSUM bank), DMA'd per tile to the [S,B,NV] output slice.
"""

import sys
from contextlib import ExitStack

for _p in ("/opt/trn_rl_repo", "/root/.axon_site/_ro/trn_rl_repo"):
    if _p not in sys.path:
        sys.path.append(_p)

import numpy as np
import ml_dtypes

import concourse.bass as bass
import concourse.tile as tile
from concourse import bacc, mybir
from concourse.bass_utils import run_bass_kernel_spmd
from concourse.masks import make_identity

VOCAB, EMB, HID = 32000, 256, 512
B, S = 16, 256
H3 = 3 * HID
NCORES = 8
NV = VOCAB // NCORES          # vocab slice per core
BF16 = mybir.dt.bfloat16
F32 = mybir.dt.float32
I32 = mybir.dt.int32
AF = mybir.ActivationFunctionType
OP = mybir.AluOpType


def build(s_len=S, nv=NV, with_bias=False, debug=False):
    """Build + compile the single-core Bass program (same NEFF on all cores)."""
    ng = s_len * B // 128              # token groups of 128
    nch = (nv + 499) // 500            # output N-chunks (<=500 = 1 PSUM bank)
    nc = bacc.Bacc("TRN2", target_bir_lowering=False, debug=debug)

    e_d = nc.dram_tensor("e_bf", [VOCAB, EMB], BF16, kind="ExternalInput")
    idx_d = nc.dram_tensor("idx", [128, ng], I32, kind="ExternalInput")
    u_d = nc.dram_tensor("u_rt", [128, 4, H3], BF16, kind="ExternalInput")
    w_d = nc.dram_tensor("w_kc", [128, EMB // 128, H3], BF16, kind="ExternalInput")
    wd_d = nc.dram_tensor("wd_rt", [128, 4, nv], BF16, kind="ExternalInput")
    if with_bias:
        bias_d = nc.dram_tensor("bias_row", [1, H3], F32, kind="ExternalInput")
        b1h_d = nc.dram_tensor("b1h_row", [1, HID], F32, kind="ExternalInput")
        bd_d = nc.dram_tensor("bd_row", [1, nv], F32, kind="ExternalInput")
    out_d = nc.dram_tensor("logits", [s_len, B, nv], F32, kind="ExternalOutput")

    with tile.TileContext(nc) as tc, ExitStack() as ctx:
        const = ctx.enter_context(tc.tile_pool(name="const", bufs=1))
        wts = ctx.enter_context(tc.tile_pool(name="wts", bufs=1))
        state = ctx.enter_context(tc.tile_pool(name="state", bufs=1))
        gates = ctx.enter_context(tc.tile_pool(name="gates", bufs=2))
        xwp = ctx.enter_context(tc.tile_pool(name="xwp", bufs=3))
        btp = ctx.enter_context(tc.tile_pool(name="btp", bufs=3))
        osb = ctx.enter_context(tc.tile_pool(name="osb", bufs=4))
        psum = ctx.enter_context(tc.tile_pool(name="ps", bufs=2, space="PSUM"))

        u_sb = wts.tile([128, 4, H3], BF16)
        nc.sync.dma_start(u_sb[:], u_d.ap()[:])
        w_sb = wts.tile([128, EMB // 128, H3], BF16)
        nc.sync.dma_start(w_sb[:], w_d.ap()[:])
        wd_sb = wts.tile([128, 4, nv], BF16)
        nc.sync.dma_start(wd_sb[:], wd_d.ap()[:])
        idx_sb = wts.tile([128, ng], I32)
        nc.sync.dma_start(idx_sb[:], idx_d.ap()[:])
        if with_bias:
            bias_sb = wts.tile([1, H3], F32)
            nc.sync.dma_start(bias_sb[:], bias_d.ap()[:])
            b1h_sb = wts.tile([1, HID], F32)
            nc.sync.dma_start(b1h_sb[:], b1h_d.ap()[:])
            bd_sb = wts.tile([1, nv], F32)
            nc.sync.dma_start(bd_sb[:], bd_d.ap()[:])

        ident = const.tile([128, 128], BF16)
        make_identity(nc, ident[:])
        # Fold selectors: sel128[r8][p, m] = 1 iff p == m + 16*r8.  A K=128
        # identity-slice matmul adds xw rows 16*r8..16*r8+16 into psum[0:16].
        sels = []
        for r8 in range(8):
            s_t = const.tile([128, 16], BF16, tag=f"sel{r8}")
            nc.gpsimd.memset(s_t[:], 0.0)
            nc.gpsimd.affine_select(
                out=s_t[:], in_=s_t[:], compare_op=OP.not_equal, fill=1.0,
                base=-16 * r8, pattern=[[-1, 16]], channel_multiplier=1)
            sels.append(s_t)

        embt = wts.tile([128, EMB // 128, s_len * B], BF16)
        h32 = state.tile([32, HID], BF16)
        nc.vector.memset(h32[:], 0.0)

        out_flat = out_d.ap().flatten_outer_dims()   # [s_len*B, nv]

        # ---- Phase A: embedding gather + transpose to embT ----
        with tc.tile_pool(name="gat", bufs=3) as gat:
            for g in range(ng):
                et = gat.tile([128, EMB], BF16, tag="emb")
                nc.gpsimd.indirect_dma_start(
                    out=et[:], out_offset=None, in_=e_d.ap()[:],
                    in_offset=bass.IndirectOffsetOnAxis(ap=idx_sb[:, g:g + 1], axis=0),
                )
                for c in range(EMB // 128):
                    pt = psum.tile([128, 128], BF16, tag="pp")
                    nc.tensor.transpose(pt[:], et[:, 128 * c:128 * c + 128], ident[:])
                    nc.vector.tensor_copy(embt[:, c, 128 * g:128 * g + 128], pt[:])

        # ---- helpers ----
        def xw_compute(g):
            """xw rows for group g's 128 tokens: [128, H3] bf16 (biases baked)."""
            xw_t = xwp.tile([128, H3], BF16, tag="xw")
            for n3 in range(3):
                pp = psum.tile([128, 512], F32, tag="pp")
                for c in range(EMB // 128):
                    nc.tensor.matmul(
                        pp[:], lhsT=embt[:, c, 128 * g:128 * g + 128],
                        rhs=w_sb[:, c, 512 * n3:512 * n3 + 512],
                        start=(c == 0), stop=(c == EMB // 128 - 1),
                    )
                if with_bias:
                    nc.vector.tensor_tensor(
                        xw_t[:, 512 * n3:512 * n3 + 512], pp[:],
                        bias_sb[:, 512 * n3:512 * n3 + 512].to_broadcast([128, 512]),
                        op=OP.add)
                else:
                    nc.vector.tensor_copy(xw_t[:, 512 * n3:512 * n3 + 512], pp[:])
            return xw_t

        def rec_mms(ps, bt_t, r8p, cols, first):
            """4 K=128-chunk matmuls accumulating h @ U[:, cols] into ps[0:16].

            bt_t is a [128, 4, 128] tile: [r, c, m] = hT[128*c + r, token m];
            r8p selects the previous step's 16 token columns.
            """
            c0, c1 = cols
            m0 = 16 * r8p
            for c in range(4):
                nc.tensor.matmul(
                    ps[:], lhsT=bt_t[:, c, m0:m0 + 16],
                    rhs=u_sb[:, c, c0:c1],
                    start=(first and c == 0), stop=(c == 3),
                )

        def emit_proj(g, n):
            """Project group g's 128 hidden states onto vocab chunk n."""
            w0 = 500 * n
            w1 = min(500 * n + 500, nv)
            nw = w1 - w0
            pp = psum.tile([128, 512], F32, tag="pp")
            bt_g = bt_tiles[g]
            for c in range(4):
                nc.tensor.matmul(
                    pp[:, 0:nw], lhsT=bt_g[:, c, :],
                    rhs=wd_sb[:, c, w0:w1],
                    start=(c == 0), stop=(c == 3),
                )
            ot = osb.tile([128, 512], F32, tag="ot")
            if with_bias:
                nc.vector.tensor_tensor(
                    ot[:, 0:nw], pp[:, 0:nw],
                    bd_sb[:, w0:w1].to_broadcast([128, nw]), op=OP.add)
            else:
                nc.vector.tensor_copy(ot[:, 0:nw], pp[:, 0:nw])
            nc.sync.dma_start(out_flat[128 * g:128 * g + 128, w0:w1], ot[:, 0:nw])

        # ---- main recurrence ----
        xw_tiles = {}
        bt_tiles = {}
        for g in range(min(2, ng)):
            xw_tiles[g] = xw_compute(g)

        for g in range(ng):
            bt_g = btp.tile([128, 4, 128], BF16, tag="bt")
            bt_tiles[g] = bt_g
            xw_g = xw_tiles.pop(g)
            for r8 in range(8):
                t = 8 * g + r8
                bt_prev = None
                if t > 0:
                    bt_prev = (bt_tiles[g], r8 - 1) if r8 > 0 \
                        else (bt_tiles[g - 1], 7)

                # PE: r-gate first (needed earliest), then h, then z.
                sel = sels[r8]
                psr = None
                psh = None
                if t > 0:
                    psr = psum.tile([16, 512], F32, tag="psr")
                    nc.tensor.matmul(psr[:], lhsT=sel[:],
                                     rhs=xw_g[:, 512:1024],
                                     start=True, stop=False)
                    rec_mms(psr, bt_prev[0], bt_prev[1], (512, 1024), first=False)
                    psh = psum.tile([16, 512], F32, tag="psh")
                    rec_mms(psh, bt_prev[0], bt_prev[1], (1024, 1536), first=True)
                # xh fold gets its own slot from the shared pp tag (rows 0:16)
                psxh = psum.tile([128, 512], F32, tag="pp")
                nc.tensor.matmul(psxh[0:16, :], lhsT=sel[:],
                                 rhs=xw_g[:, 1024:1536],
                                 start=True, stop=True)
                psz = psum.tile([16, 512], F32, tag="psz")
                nc.tensor.matmul(psz[:], lhsT=sel[:],
                                 rhs=xw_g[:, 0:512],
                                 start=True, stop=(t == 0))
                if t > 0:
                    rec_mms(psz, bt_prev[0], bt_prev[1], (0, 512),
                            first=False)

                # gates
                if t > 0:
                    r_t = gates.tile([16, 512], F32, tag="r")
                    nc.scalar.activation(r_t[:], psr[:], AF.Sigmoid)
                z_t = gates.tile([16, 512], F32, tag="z")
                nc.scalar.activation(z_t[:], psz[:], AF.Sigmoid)
                hh = gates.tile([16, 512], F32, tag="hh")
                if t > 0:
                    t1 = gates.tile([16, 512], F32, tag="t1")
                    if with_bias:
                        tb = gates.tile([16, 512], F32, tag="tb")
                        nc.vector.tensor_tensor(
                            tb[:], psh[:], b1h_sb[:].to_broadcast([16, HID]),
                            op=OP.add)
                        nc.vector.tensor_tensor(t1[:], r_t[:], tb[:], op=OP.mult)
                    else:
                        nc.vector.tensor_tensor(t1[:], r_t[:], psh[:],
                                                op=OP.mult)
                    t2 = gates.tile([16, 512], F32, tag="t2")
                    nc.vector.tensor_tensor(t2[:], t1[:], psxh[0:16, :],
                                            op=OP.add)
                    nc.scalar.activation(hh[:], t2[:], AF.Tanh)
                else:
                    nc.scalar.activation(hh[:], psxh[0:16, :], AF.Tanh)

                # h_new = hh + z*(h - hh)
                d_t = gates.tile([16, 512], F32, tag="d")
                nc.gpsimd.tensor_sub(d_t[:], h32[0:16, :], hh[:])
                e_t = gates.tile([16, 512], F32, tag="e")
                nc.gpsimd.tensor_mul(e_t[:], d_t[:], z_t[:])
                nc.vector.tensor_add(h32[0:16, :], e_t[:], hh[:])

                # block-transpose h, then scatter blocks into hT layout:
                # btq[32j+p, c, m0+q] = btt[p, 32*(4c+j)+q] = hT[128c+32j+p, tok]
                btt = gates.tile([32, HID], BF16, tag="btt")
                nc.vector.transpose(btt[:], h32[:, :])
                btt_v = btt[:].rearrange("p (c j q) -> p c j q", c=4, q=32)
                m0 = 16 * r8
                nc.gpsimd.tensor_copy(bt_g[0:32, :, m0:m0 + 16],
                                      btt_v[:, :, 0, 0:16])
                nc.gpsimd.tensor_copy(bt_g[32:64, :, m0:m0 + 16],
                                      btt_v[:, :, 1, 0:16])
                nc.vector.tensor_copy(bt_g[64:96, :, m0:m0 + 16],
                                      btt_v[:, :, 2, 0:16])
                nc.vector.tensor_copy(bt_g[96:128, :, m0:m0 + 16],
                                      btt_v[:, :, 3, 0:16])

                # interleave previous group's projection chunks
                if g > 0:
                    for n in range(nch):
                        if n * 8 // nch == r8:
                            emit_proj(g - 1, n)
            if g + 2 < ng:
                xw_tiles[g + 2] = xw_compute(g + 2)

        for n in range(nch):
            emit_proj(ng - 1, n)

    nc.compile()
    return nc


_NC_CACHE = {}


def _get_nc(key):
    if key not in _NC_CACHE:
        _NC_CACHE[key] = build(with_bias=key)
    return _NC_CACHE[key]


def prep_inputs(x, E, W, U, b, Wd, bd, s_len=S, nv=NV, ncores=NCORES):
    """Host-side layout prep.  Returns (in_maps, with_bias)."""
    bf = ml_dtypes.bfloat16
    x = np.asarray(x, np.int32)
    E = np.asarray(E, np.float32)
    W = np.asarray(W, np.float32)
    U = np.asarray(U, np.float32)
    b = np.asarray(b, np.float32)
    Wd = np.asarray(Wd, np.float32)
    bd = np.asarray(bd, np.float32)

    ng = s_len * B // 128
    # s-major token ids -> [128, ng]
    arr = x[:, :s_len].T.reshape(-1)                      # t' = s*16+b
    idx = np.ascontiguousarray(arr.reshape(ng, 128).T)    # [128, ng]
    e_bf = E.astype(bf)
    u_rt = np.ascontiguousarray(
        U.reshape(4, 128, H3).transpose(1, 0, 2)).astype(bf)
    w_kc = np.ascontiguousarray(
        W.reshape(EMB // 128, 128, H3).transpose(1, 0, 2)).astype(bf)

    with_bias = bool(np.any(b) or np.any(bd))
    common = {"e_bf": e_bf, "idx": idx, "u_rt": u_rt, "w_kc": w_kc}
    if with_bias:
        bias_row = (b[0] + b[1]).astype(np.float32).copy()
        bias_row[2 * HID:] = b[0, 2 * HID:]               # h-block: b0 only
        common["bias_row"] = bias_row.reshape(1, H3)
        common["b1h_row"] = b[1, 2 * HID:].astype(np.float32).reshape(1, HID)

    in_maps = []
    for c in range(ncores):
        wd_slice = Wd[:, c * nv:(c + 1) * nv]
        wd_rt = np.ascontiguousarray(
            wd_slice.reshape(4, 128, nv).transpose(1, 0, 2)).astype(bf)
        m = dict(common)
        m["wd_rt"] = wd_rt
        if with_bias:
            m["bd_row"] = bd[c * nv:(c + 1) * nv].astype(np.float32).reshape(1, nv)
        in_maps.append(m)
    return in_maps, with_bias


def assemble_output(results, s_len=S, nv=NV):
    """Per-core [s_len, B, nv] slices -> [B, s_len, VOCAB]."""
    return np.concatenate(
        [np.asarray(r["logits"]).transpose(1, 0, 2) for r in results], axis=2)


def kernel(x, E, W, U, b, Wd, bd):
    in_maps, with_bias = prep_inputs(x, E, W, U, b, Wd, bd)
    nc = _get_nc(with_bias)
    res = run_bass_kernel_spmd(nc, in_maps, core_ids=list(range(NCORES)))
    return assemble_output(res.results)


# revision 21
# speedup vs baseline: 1.2476x; 1.2476x over previous
"""Trainium2 Bass kernel for a GRU language model.

  logits[b,s,:] = Dense(GRU(Embed(x)))   (keras GRU, reset_after=True)

Shapes: x[16,256] i32, E[32000,256], W[256,1536], U[512,1536], b[2,1536],
Wd[512,32000], bd[32000] -> logits [16,256,32000] f32.

Strategy (8 cores):
  * Tensor-parallel over the vocab dim of Wd (4000 cols/core); the GRU
    recurrence is replicated on every core (it is serial in S and cheap
    compared to the 524MB logits write).  No collectives.
  * s-major token order: token t' = s*16 + b.  Each "group" g covers 128
    tokens = 8 timesteps x 16 batch.
  * Embedding gather via indirect DMA (128 rows/descriptor-batch).
  * xw = emb @ W precomputed per group on PE (embT built once via PE
    transposes); folded into the per-step PSUM with an identity matmul.
  * Recurrence h @ U: the hidden state is kept in DVE 32x32
    block-transposed form (bt), replicated to the 4 partition row-groups,
    so each step's matmuls are 4-way row-tiled (K=32 tiles at partition
    offsets 0/32/64/96) which the PE runs concurrently.
  * All matmul operands bf16, accumulation fp32 in PSUM.
  * Output projection uses the same row-tiled trick straight out of the
    stored bt tiles (lhsT = bt[32i:32i+32, 0:8, 32b:32b+16]), N-chunks of
    500 (one PSUM bank), DMA'd per tile to the [S,B,NV] output slice.
"""

import sys
from contextlib import ExitStack

for _p in ("/opt/trn_rl_repo", "/root/.axon_site/_ro/trn_rl_repo"):
    if _p not in sys.path:
        sys.path.append(_p)

import numpy as np
import ml_dtypes

import concourse.bass as bass
import concourse.tile as tile
from concourse import bacc, mybir
from concourse.bass_utils import run_bass_kernel_spmd
from concourse.masks import make_identity

VOCAB, EMB, HID = 32000, 256, 512
B, S = 16, 256
H3 = 3 * HID
NCORES = 8
NV = VOCAB // NCORES          # vocab slice per core
BF16 = mybir.dt.bfloat16
F32 = mybir.dt.float32
I32 = mybir.dt.int32
AF = mybir.ActivationFunctionType
OP = mybir.AluOpType


def build(s_len=S, nv=NV, with_bias=False, debug=False):
    """Build + compile the single-core Bass program (same NEFF on all cores)."""
    ng = s_len * B // 128              # token groups of 128
    nch = (nv + 499) // 500            # output N-chunks (<=500 = 1 PSUM bank)
    nc = bacc.Bacc("TRN2", target_bir_lowering=False, debug=debug)

    e_d = nc.dram_tensor("e_bf", [VOCAB, EMB], BF16, kind="ExternalInput")
    idx_d = nc.dram_tensor("idx", [128, ng], I32, kind="ExternalInput")
    u_d = nc.dram_tensor("u_rt", [128, 4, H3], BF16, kind="ExternalInput")
    w_d = nc.dram_tensor("w_kc", [128, EMB // 128, H3], BF16, kind="ExternalInput")
    wd_d = nc.dram_tensor("wd_rt", [128, 4, nv], BF16, kind="ExternalInput")
    if with_bias:
        bias_d = nc.dram_tensor("bias_row", [1, H3], F32, kind="ExternalInput")
        b1h_d = nc.dram_tensor("b1h_row", [1, HID], F32, kind="ExternalInput")
        bd_d = nc.dram_tensor("bd_row", [1, nv], F32, kind="ExternalInput")
    out_d = nc.dram_tensor("logits", [s_len, B, nv], F32, kind="ExternalOutput")

    with tile.TileContext(nc) as tc, ExitStack() as ctx:
        const = ctx.enter_context(tc.tile_pool(name="const", bufs=1))
        wts = ctx.enter_context(tc.tile_pool(name="wts", bufs=1))
        state = ctx.enter_context(tc.tile_pool(name="state", bufs=1))
        gates = ctx.enter_context(tc.tile_pool(name="gates", bufs=2))
        xwp = ctx.enter_context(tc.tile_pool(name="xwp", bufs=3))
        btp = ctx.enter_context(tc.tile_pool(name="btp", bufs=3))
        osb = ctx.enter_context(tc.tile_pool(name="osb", bufs=4))
        psum = ctx.enter_context(tc.tile_pool(name="ps", bufs=2, space="PSUM"))

        u_sb = wts.tile([128, 4, H3], BF16)
        nc.sync.dma_start(u_sb[:], u_d.ap()[:])
        w_sb = wts.tile([128, EMB // 128, H3], BF16)
        nc.sync.dma_start(w_sb[:], w_d.ap()[:])
        wd_sb = wts.tile([128, 4, nv], BF16)
        nc.sync.dma_start(wd_sb[:], wd_d.ap()[:])
        idx_sb = wts.tile([128, ng], I32)
        nc.sync.dma_start(idx_sb[:], idx_d.ap()[:])
        if with_bias:
            bias_sb = wts.tile([1, H3], F32)
            nc.sync.dma_start(bias_sb[:], bias_d.ap()[:])
            b1h_sb = wts.tile([1, HID], F32)
            nc.sync.dma_start(b1h_sb[:], b1h_d.ap()[:])
            bd_sb = wts.tile([1, nv], F32)
            nc.sync.dma_start(bd_sb[:], bd_d.ap()[:])

        ident = const.tile([128, 128], BF16)
        make_identity(nc, ident[:])
        # Fold selectors: sel128[r8][p, m] = 1 iff p == m + 16*r8.  A K=128
        # identity-slice matmul adds xw rows 16*r8..16*r8+16 into psum[0:16].
        sels = []
        for r8 in range(8):
            s_t = const.tile([128, 16], BF16, tag=f"sel{r8}")
            nc.gpsimd.memset(s_t[:], 0.0)
            nc.gpsimd.affine_select(
                out=s_t[:], in_=s_t[:], compare_op=OP.not_equal, fill=1.0,
                base=-16 * r8, pattern=[[-1, 16]], channel_multiplier=1)
            sels.append(s_t)

        embt = wts.tile([128, EMB // 128, s_len * B], BF16)
        h32 = state.tile([32, HID], BF16)
        nc.vector.memset(h32[:], 0.0)

        out_flat = out_d.ap().flatten_outer_dims()   # [s_len*B, nv]

        # ---- Phase A: embedding gather + transpose to embT ----
        with tc.tile_pool(name="gat", bufs=3) as gat:
            for g in range(ng):
                et = gat.tile([128, EMB], BF16, tag="emb")
                nc.gpsimd.indirect_dma_start(
                    out=et[:], out_offset=None, in_=e_d.ap()[:],
                    in_offset=bass.IndirectOffsetOnAxis(ap=idx_sb[:, g:g + 1], axis=0),
                )
                for c in range(EMB // 128):
                    pt = psum.tile([128, 128], BF16, tag="pp")
                    nc.tensor.transpose(pt[:], et[:, 128 * c:128 * c + 128], ident[:])
                    nc.vector.tensor_copy(embt[:, c, 128 * g:128 * g + 128], pt[:])

        # ---- helpers ----
        def xw_compute(g):
            """xw rows for group g's 128 tokens: [128, H3] bf16 (biases baked)."""
            xw_t = xwp.tile([128, H3], BF16, tag="xw")
            for n3 in range(3):
                pp = psum.tile([128, 512], F32, tag="pp")
                for c in range(EMB // 128):
                    nc.tensor.matmul(
                        pp[:], lhsT=embt[:, c, 128 * g:128 * g + 128],
                        rhs=w_sb[:, c, 512 * n3:512 * n3 + 512],
                        start=(c == 0), stop=(c == EMB // 128 - 1),
                    )
                if with_bias:
                    nc.vector.tensor_tensor(
                        xw_t[:, 512 * n3:512 * n3 + 512], pp[:],
                        bias_sb[:, 512 * n3:512 * n3 + 512].to_broadcast([128, 512]),
                        op=OP.add)
                else:
                    nc.vector.tensor_copy(xw_t[:, 512 * n3:512 * n3 + 512], pp[:])
            return xw_t

        def rec_mms(ps, bt_t, r8p, cols, first):
            """4 K=128-chunk matmuls accumulating h @ U[:, cols] into ps[0:16].

            bt_t is a [128, 4, 128] tile: [r, c, m] = hT[128*c + r, token m];
            r8p selects the previous step's 16 token columns.
            """
            c0, c1 = cols
            m0 = 16 * r8p
            for c in range(4):
                nc.tensor.matmul(
                    ps[:], lhsT=bt_t[:, c, m0:m0 + 16],
                    rhs=u_sb[:, c, c0:c1],
                    start=(first and c == 0), stop=(c == 3),
                )

        def emit_proj(g, n):
            """Project group g's 128 hidden states onto vocab chunk n."""
            w0 = 500 * n
            w1 = min(500 * n + 500, nv)
            nw = w1 - w0
            pp = psum.tile([128, 512], F32, tag="pp")
            bt_g = bt_tiles[g]
            for c in range(4):
                nc.tensor.matmul(
                    pp[:, 0:nw], lhsT=bt_g[:, c, :],
                    rhs=wd_sb[:, c, w0:w1],
                    start=(c == 0), stop=(c == 3),
                )
            ot = osb.tile([128, 512], F32, tag="ot")
            if with_bias:
                nc.vector.tensor_tensor(
                    ot[:, 0:nw], pp[:, 0:nw],
                    bd_sb[:, w0:w1].to_broadcast([128, nw]), op=OP.add)
            else:
                nc.vector.tensor_copy(ot[:, 0:nw], pp[:, 0:nw])
            nc.sync.dma_start(out_flat[128 * g:128 * g + 128, w0:w1], ot[:, 0:nw])

        # ---- main recurrence ----
        xw_tiles = {}
        bt_tiles = {}
        for g in range(min(2, ng)):
            xw_tiles[g] = xw_compute(g)

        for g in range(ng):
            bt_g = btp.tile([128, 4, 128], BF16, tag="bt")
            bt_tiles[g] = bt_g
            xw_g = xw_tiles.pop(g)
            for r8 in range(8):
                t = 8 * g + r8
                m0 = 16 * r8
                bt_prev = None
                if t > 0:
                    bt_prev = (bt_tiles[g], r8 - 1) if r8 > 0 \
                        else (bt_tiles[g - 1], 7)

                # The step is split into two 256-column halves of the hidden
                # dim so half 0's gate chain overlaps half 1's (and the next
                # step's) PE work.  z is computed full-width once (it is off
                # the latency-critical r->hh path).
                sel = sels[r8]
                psr = psh = None
                psxh = psum.tile([128, 512], F32, tag="pp")
                psz = psum.tile([16, 512], F32, tag="psz")
                if t > 0:
                    psr = psum.tile([16, 512], F32, tag="psr")
                    psh = psum.tile([16, 512], F32, tag="psh")
                b0, b1 = bt_prev if t > 0 else (None, None)
                mp = 16 * bt_prev[1] if t > 0 else 0

                def half_mms(v):
                    h0 = 256 * v
                    hs = slice(h0, h0 + 256)
                    nc.tensor.matmul(psxh[0:16, hs], lhsT=sel[:],
                                     rhs=xw_g[:, 1024 + h0:1280 + h0],
                                     start=True, stop=True)
                    if t == 0:
                        return
                    nc.tensor.matmul(psr[:, hs], lhsT=sel[:],
                                     rhs=xw_g[:, 512 + h0:768 + h0],
                                     start=True, stop=False)
                    for c in range(4):
                        nc.tensor.matmul(
                            psr[:, hs], lhsT=b0[:, c, mp:mp + 16],
                            rhs=u_sb[:, c, 512 + h0:768 + h0],
                            start=False, stop=(c == 3))
                        nc.tensor.matmul(
                            psh[:, hs], lhsT=b0[:, c, mp:mp + 16],
                            rhs=u_sb[:, c, 1024 + h0:1280 + h0],
                            start=(c == 0), stop=(c == 3))

                def z_mms():
                    nc.tensor.matmul(psz[:], lhsT=sel[:], rhs=xw_g[:, 0:512],
                                     start=True, stop=(t == 0))
                    if t > 0:
                        for c in range(4):
                            nc.tensor.matmul(
                                psz[:], lhsT=b0[:, c, mp:mp + 16],
                                rhs=u_sb[:, c, 0:512],
                                start=False, stop=(c == 3))

                half_mms(0)
                z_mms()
                half_mms(1)

                # z path (full width, off critical chain)
                z_t = gates.tile([16, 512], F32, tag="z")
                nc.scalar.activation(z_t[:], psz[:], AF.Sigmoid)
                zc = gates.tile([16, 512], F32, tag="zc")
                nc.gpsimd.tensor_scalar(zc[:], z_t[:], -1.0, 1.0,
                                        OP.mult, OP.add)
                e1 = gates.tile([16, 512], F32, tag="e1")
                nc.gpsimd.tensor_mul(e1[:], z_t[:], h32[0:16, :])

                for v in range(2):
                    h0 = 256 * v
                    hs = slice(h0, h0 + 256)
                    hh = gates.tile([16, 256], F32, tag=f"hh{v}")
                    if t > 0:
                        r_t = gates.tile([16, 256], F32, tag=f"r{v}")
                        nc.scalar.activation(r_t[:], psr[:, hs], AF.Sigmoid)
                        t1 = gates.tile([16, 256], F32, tag=f"t1{v}")
                        if with_bias:
                            tb = gates.tile([16, 256], F32, tag=f"tb{v}")
                            nc.vector.tensor_tensor(
                                tb[:], psh[:, hs],
                                b1h_sb[:, hs].to_broadcast([16, 256]), op=OP.add)
                            nc.vector.tensor_tensor(t1[:], r_t[:], tb[:],
                                                    op=OP.mult)
                        else:
                            nc.vector.tensor_tensor(t1[:], r_t[:], psh[:, hs],
                                                    op=OP.mult)
                        t2 = gates.tile([16, 256], F32, tag=f"t2{v}")
                        nc.vector.tensor_tensor(t2[:], t1[:], psxh[0:16, hs],
                                                op=OP.add)
                        nc.scalar.activation(hh[:], t2[:], AF.Tanh)
                    else:
                        nc.scalar.activation(hh[:], psxh[0:16, hs], AF.Tanh)
                    # h_new = z*h_old + (1-z)*hh
                    m2 = gates.tile([16, 256], F32, tag=f"m2{v}")
                    nc.vector.tensor_tensor(m2[:], zc[0:16, hs], hh[:],
                                            op=OP.mult)
                    nc.vector.tensor_add(h32[0:16, hs], e1[0:16, hs], m2[:])
                    # transpose this half and scatter into btq chunks 2v, 2v+1
                    btt = gates.tile([32, 256], BF16, tag=f"btt{v}")
                    nc.vector.transpose(btt[:], h32[:, hs])
                    bv = btt[:].rearrange("p (c j q) -> p c j q", c=2, q=32)
                    c0, c1 = 2 * v, 2 * v + 2
                    nc.gpsimd.tensor_copy(bt_g[0:32, c0:c1, m0:m0 + 16],
                                          bv[:, :, 0, 0:16])
                    nc.gpsimd.tensor_copy(bt_g[32:64, c0:c1, m0:m0 + 16],
                                          bv[:, :, 1, 0:16])
                    nc.vector.tensor_copy(bt_g[64:96, c0:c1, m0:m0 + 16],
                                          bv[:, :, 2, 0:16])
                    nc.vector.tensor_copy(bt_g[96:128, c0:c1, m0:m0 + 16],
                                          bv[:, :, 3, 0:16])

                # interleave previous group's projection chunks
                if g > 0:
                    for n in range(nch):
                        if n * 8 // nch == r8:
                            emit_proj(g - 1, n)
            if g + 2 < ng:
                xw_tiles[g + 2] = xw_compute(g + 2)

        for n in range(nch):
            emit_proj(ng - 1, n)

    nc.compile()
    return nc


_NC_CACHE = {}


def _get_nc(key):
    if key not in _NC_CACHE:
        _NC_CACHE[key] = build(with_bias=key)
    return _NC_CACHE[key]


def prep_inputs(x, E, W, U, b, Wd, bd, s_len=S, nv=NV, ncores=NCORES):
    """Host-side layout prep.  Returns (in_maps, with_bias)."""
    bf = ml_dtypes.bfloat16
    x = np.asarray(x, np.int32)
    E = np.asarray(E, np.float32)
    W = np.asarray(W, np.float32)
    U = np.asarray(U, np.float32)
    b = np.asarray(b, np.float32)
    Wd = np.asarray(Wd, np.float32)
    bd = np.asarray(bd, np.float32)

    ng = s_len * B // 128
    # s-major token ids -> [128, ng]
    arr = x[:, :s_len].T.reshape(-1)                      # t' = s*16+b
    idx = np.ascontiguousarray(arr.reshape(ng, 128).T)    # [128, ng]
    e_bf = E.astype(bf)
    u_rt = np.ascontiguousarray(
        U.reshape(4, 128, H3).transpose(1, 0, 2)).astype(bf)
    w_kc = np.ascontiguousarray(
        W.reshape(EMB // 128, 128, H3).transpose(1, 0, 2)).astype(bf)

    with_bias = bool(np.any(b) or np.any(bd))
    common = {"e_bf": e_bf, "idx": idx, "u_rt": u_rt, "w_kc": w_kc}
    if with_bias:
        bias_row = (b[0] + b[1]).astype(np.float32).copy()
        bias_row[2 * HID:] = b[0, 2 * HID:]               # h-block: b0 only
        common["bias_row"] = bias_row.reshape(1, H3)
        common["b1h_row"] = b[1, 2 * HID:].astype(np.float32).reshape(1, HID)

    in_maps = []
    for c in range(ncores):
        wd_slice = Wd[:, c * nv:(c + 1) * nv]
        wd_rt = np.ascontiguousarray(
            wd_slice.reshape(4, 128, nv).transpose(1, 0, 2)).astype(bf)
        m = dict(common)
        m["wd_rt"] = wd_rt
        if with_bias:
            m["bd_row"] = bd[c * nv:(c + 1) * nv].astype(np.float32).reshape(1, nv)
        in_maps.append(m)
    return in_maps, with_bias


def assemble_output(results, s_len=S, nv=NV):
    """Per-core [s_len, B, nv] slices -> [B, s_len, VOCAB]."""
    return np.concatenate(
        [np.asarray(r["logits"]).transpose(1, 0, 2) for r in results], axis=2)


def kernel(x, E, W, U, b, Wd, bd):
    in_maps, with_bias = prep_inputs(x, E, W, U, b, Wd, bd)
    nc = _get_nc(with_bias)
    res = run_bass_kernel_spmd(nc, in_maps, core_ids=list(range(NCORES)))
    return assemble_output(res.results)
